# revision 1
# baseline (speedup 1.0000x reference)
"""Distillation loss (chunked KL + CE) on 8 Trainium2 NeuronCores — v5.

Probe-driven design (see probe.py phases 1-9). On this axon terminal the
HW behaves nothing like the instruction cost model:

  * instruction slots cost ~17-100us each (load-dependent), roughly
    serialized PER CORE across all engines; streaming afterwards runs at
    ~210-260 G elem/s, so wide ops are nearly free beyond the slot cost;
  * DMA costs ~0.2-0.9us per descriptor (one per contiguous run per
    partition, 64KB max per MAX_DMA_LAST_DIM).  The original kernel's
    8192 x 16KB descriptors are exactly its 5.9ms baseline.

v5 minimizes total instruction slots (19/core, zero standalone waits —
every dependency is transitively implied by one fusable wait per op):

  * fp8(e3m4) transport, host pre-scales logits by 1/4 so every device
    intermediate ((t-s)/4, e*(t-s)/4, exp(s)/32) stays in fp8 range;
  * host concatenates both logit tensors into ts=[2, 512, 32000]; ONE
    dma_start per half-shard loads t->tb and s->sb with 2 x 64KB
    descriptors per partition (512 descriptors/core total);
  * SBUF arena [128, 192KB] = tb | eb | sb (adjacent, so one
    tensor_reduce can span two buffers);
  * per half-shard (256 tokens = [128 part, 2 tok, 32000 vocab]):
      ACT A1 : eb = exp(0.8*tb)             e_t       (0.8*(t/4)=t/5)
      GP  G1 : tb = tb - sb                 D = (t-s)/4
      GP  G2 : tb = tb * eb                 P = e_t*D
      DVE R1 : reduce(tb|eb [128,16,8000]) -> [W(8) | Zu(8)]
      ACT A2 : eb = exp(0.8*sb)             e_s
      ACT A3 : tb = exp(4*sb - log 32)      e_ce' = exp(s)/32  (tb dead)
      DVE R2 : reduce(tb|eb [128,16,8000]) -> [Zce'(8) | Zv(8)]
  * host combine (float64):
      kl  = 4*W/(T*Zu) + log Zv - log Zu    per (token, chunk)
      zce = 32 * sum_chunks Zce'            -> ce with host s[label] gather.
"""

import math
from contextlib import ExitStack

import numpy as np

import concourse.bass as bass
import concourse.mybir as mybir
from concourse.bass_utils import run_bass_kernel_spmd

ALPHA = 0.7
TEMP = 5.0
PAD_ID = 0
NUM_CHUNKS = 4

N_CORES = 8
B, S, V = 2, 2048, 32000
TOK = B * S                      # 4096 tokens total
TPC = TOK // N_CORES             # 512 tokens per core
P = 128                          # SBUF partitions
TPP = 2                          # tokens per partition per tile
HALVES = TPC // (P * TPP)        # 2 half-shards per core
CHW = V // NUM_CHUNKS            # 8000
PRESCALE = 0.25                  # host multiplies logits by this before fp8
CE_SCALE = 32.0                  # e_ce stored as exp(s)/32 to fit e3m4
S_OFF = math.log(CE_SCALE) / 4.0  # host subtracts from s/4; exp(4*s'')=exp(s)/32

F8 = mybir.dt.float8e3           # e3m4: max 15.5
F32 = mybir.dt.float32
EXP = mybir.ActivationFunctionType.Exp
MULT = mybir.AluOpType.mult
SUB = mybir.AluOpType.subtract
ADD = mybir.AluOpType.add
X = mybir.AxisListType.X

WF = TPP * V                     # 64000 elems per partition per buffer
NSTAT = 32                       # per half: W[8] Zu[8] Zce'[8] Zv[8]


def _build_nc(repeat=1):
    """Per-core program over ts=[2, TPC, V] fp8 (pre-scaled by 1/4)."""
    nc = bass.Bass()
    ts = nc.dram_tensor("ts", [2, TPC, V], F8, kind="ExternalInput")
    st = nc.dram_tensor("stats", [P, 2 * NSTAT], F32, kind="ExternalOutput")

    with ExitStack() as ctx:
        arena = ctx.enter_context(nc.sbuf_tensor("arena", [P, 3 * WF], F8))
        tb = arena[:, 0:WF]
        eb = arena[:, WF:2 * WF]
        sb = arena[:, 2 * WF:3 * WF]
        acc = ctx.enter_context(nc.sbuf_tensor("acc", [P, 2 * NSTAT], F32))
        ceb = ctx.enter_context(nc.sbuf_tensor("ceb", [P, 1], F32))
        dTS = ctx.enter_context(nc.semaphore("dTS"))  # +16 per merged load
        aE = ctx.enter_context(nc.semaphore("aE"))    # +3 per iter (ACT)
        gP = ctx.enter_context(nc.semaphore("gP"))    # +2 per iter (GPSIMD)
        vD = ctx.enter_context(nc.semaphore("vD"))    # +2 per iter (DVE)
        out_sem = ctx.enter_context(nc.semaphore("out_sem"))
        block = ctx.enter_context(nc.Block())

        niter = HALVES * repeat

        # Dependency ledger (it = iteration index, all waits fuse):
        #  DMA(it):  vD >= 2it        (R2(it-1): last reader of tb AND sb)
        #  A1(it):   dTS >= 16(it+1)  (loads; implies vD>=2it via DMA's wait)
        #  G1(it):   aE >= 3it+1      (A1 read tb; implies loads)
        #  G2(it):   -                (program order after G1)
        #  R1(it):   gP >= 2it+2      (G2 wrote P; implies A1 wrote e_t)
        #  A2(it):   vD >= 2it+1      (R1 read eb/tb; implies G2 freed eb)
        #  A3(it):   -                (after A2: implies R1 freed tb, G1
        #                              done with sb; A2 read sb)
        #  R2(it):   aE >= 3it+3      (A3; implies A2 wrote e_s)

        @block.sync
        def _(sync):
            for it in range(niter):
                r0 = (it % HALVES) * P * TPP
                if it > 0:
                    sync.wait_ge(vD, 2 * it)
                src = ts[:, r0:r0 + P * TPP, :].rearrange(
                    "a (p j) v -> p a (j v)", p=P)
                dst = arena[:].rearrange("p (b x) -> p b x", b=3)[:, 0:3:2, :]
                sync.dma_start(out=dst, in_=src).then_inc(dTS, 16)
            sync.wait_ge(vD, 2 * niter)
            sync.dma_start(out=st[:, :], in_=acc[:]).then_inc(out_sem, 16)
            sync.wait_ge(out_sem, 16)

        @block.scalar
        def _(scalar):
            for it in range(niter):
                scalar.wait_ge(dTS, 16 * (it + 1))
                nc.scalar.activation(
                    eb, tb, EXP, bias=0.0, scale=0.8,
                ).then_inc(aE, 1)
                scalar.wait_ge(vD, 2 * it + 1)
                nc.scalar.activation(
                    eb, sb, EXP, bias=0.0, scale=0.8,
                ).then_inc(aE, 1)
                nc.scalar.activation(
                    tb, sb, EXP, bias=ceb[:, 0:1], scale=4.0,
                ).then_inc(aE, 1)

        @block.gpsimd
        def _(gp):
            # bias for A3 (= -log CE_SCALE), ready before any A3 via the
            # A3 <- A2 <- R1 <- G2 <- (program order) memset chain.
            nc.gpsimd.memset(ceb[:, 0:1], -math.log(CE_SCALE))
            for it in range(niter):
                gp.wait_ge(aE, 3 * it + 1)
                nc.gpsimd.tensor_tensor(
                    out=tb, in0=tb, in1=sb, op=SUB,
                ).then_inc(gP, 1)
                nc.gpsimd.tensor_tensor(
                    out=tb, in0=tb, in1=eb, op=MULT,
                ).then_inc(gP, 1)

        @block.vector
        def _(vector):
            for it in range(niter):
                o = NSTAT * (it % HALVES)
                tbeb = arena[:, 0:2 * WF].rearrange("p (m x) -> p m x", x=CHW)
                # R1 -> [W(8) | Zu(8)]
                vector.wait_ge(gP, 2 * it + 2)
                nc.vector.tensor_reduce(
                    out=acc[:, o + 0:o + 16], in_=tbeb, axis=X, op=ADD,
                ).then_inc(vD, 1)
                # R2 -> [Zce'(8) | Zv(8)]
                vector.wait_ge(aE, 3 * it + 3)
                nc.vector.tensor_reduce(
                    out=acc[:, o + 16:o + 32], in_=tbeb, axis=X, op=ADD,
                ).then_inc(vD, 1)

    return nc


_NC_CACHE = {}
last_results = None


def _get_nc(repeat=1):
    if repeat not in _NC_CACHE:
        _NC_CACHE[repeat] = _build_nc(repeat)
    return _NC_CACHE[repeat]


def _combine(results, s_full, lab):
    """Host-side float64 reduction of per-core [128, 64] stats -> loss."""
    # token index = core*TPC + 256*h + 2*p + j
    zu = np.empty((TOK, NUM_CHUNKS))
    w = np.empty((TOK, NUM_CHUNKS))
    zv = np.empty((TOK, NUM_CHUNKS))
    zce = np.empty(TOK)
    for c, r in enumerate(results):
        a = r["stats"].astype(np.float64)          # [128, 64]
        for h in range(HALVES):
            o = NSTAT * h
            base = c * TPC + P * TPP * h
            w_h = a[:, o + 0:o + 8].reshape(P, TPP, NUM_CHUNKS)
            zu_h = a[:, o + 8:o + 16].reshape(P, TPP, NUM_CHUNKS)
            zc_h = a[:, o + 16:o + 24].reshape(P, TPP, NUM_CHUNKS)
            zv_h = a[:, o + 24:o + 32].reshape(P, TPP, NUM_CHUNKS)
            idx = (base + 2 * np.arange(P)[:, None]
                   + np.arange(TPP)[None, :]).ravel()
            w[idx] = w_h.reshape(-1, NUM_CHUNKS)
            zu[idx] = zu_h.reshape(-1, NUM_CHUNKS)
            zv[idx] = zv_h.reshape(-1, NUM_CHUNKS)
            zce[idx] = CE_SCALE * zc_h.reshape(-1, NUM_CHUNKS).sum(axis=1)

    # W stored = sum e_t*(t-s)/4 -> true sum e_t*(t-s) = 4*W
    kl = (4.0 * w) / (TEMP * zu) + np.log(zv) - np.log(zu)
    total_kl = kl.sum() * (TEMP * TEMP) * (CHW / V) / B

    s_label = s_full[np.arange(TOK), lab].astype(np.float64)
    nll = np.log(zce) - s_label
    valid = lab != PAD_ID
    n_valid = max(int(valid.sum()), 1)
    ce = float(nll[valid].sum()) / n_valid

    return ALPHA * total_kl + (1.0 - ALPHA) * ce


def kernel(student_logits, teacher_logits, labels):
    global last_results
    np_f8 = mybir.dt.np(F8)
    s_full = np.asarray(student_logits, dtype=np.float32).reshape(TOK, V)
    t_full = np.asarray(teacher_logits, dtype=np.float32).reshape(TOK, V)
    lab = np.asarray(labels).reshape(TOK).astype(np.int64)
    s_f8 = (s_full * PRESCALE).astype(np_f8)
    t_f8 = (t_full * PRESCALE).astype(np_f8)

    nc = _get_nc()
    in_maps = []
    for c in range(N_CORES):
        ts = np.ascontiguousarray(np.stack(
            [t_f8[c * TPC:(c + 1) * TPC], s_f8[c * TPC:(c + 1) * TPC]], axis=0))
        in_maps.append({"ts": ts})
    last_results = run_bass_kernel_spmd(nc, in_maps, core_ids=list(range(N_CORES)))
    loss = _combine(last_results.results, s_full, lab)
    return np.array(loss, dtype=np.float32)



# revision 3
# speedup vs baseline: 3.9931x; 3.9931x over previous
"""Distillation loss (chunked KL + CE) on 8 Trainium2 NeuronCores — v6.

v5 (1.46ms cost-model timeline) was fully serialized: every wide op
depended on the previous one through a shared 3-buffer arena, so the
per-core time was the SUM of all engine busy times (ACT 320us + Pool
508us + DVE 534us + DMA 91us).

v6 restructures the math so the per-(token,chunk) sums come for free
from instruction-level accumulators, drops GPSIMD entirely, and
pipelines quarter-shard tiles so the Activation engine (the irreducible
3x exp streams) is the only wall:

  * per core: ts=[2, 512, 32000] fp8 (host pre-scales logits by 1/4);
    4 tiles of 128 tokens, one token per partition (full 32000-vocab row
    contiguous per partition), double-buffered DMA (2x32000B descriptors
    per partition per tile).
  * per segment g = (tile q, chunk k) = [128, 8000]:
      ACT A1: et = exp(0.8*t')        accum -> Zu(g)   (e_t = exp(t/5))
      ACT A2: junk = exp(0.8*s')      accum -> Zv(g)
      ACT A3: junk = exp(4*s') bf16   accum -> Zce(g)  (= exp(s), no
              bias needed: bf16 out cannot overflow like fp8)
      DVE D : db = t' - s'            (= (t-s)/4, fp8)
      DVE W : stt (db bypass 1) mult et, accum -> W'(g) (= sum e_t(t-s)/4)
  * accumulators are f32 sums of the pre-downcast activation outputs --
    strictly more precise than v5's fp8 tensor_reduce.
  * engine busy/core: ACT 336us (wall), DVE 271us, DMA 91us, Pool/PE 0.
  * host combine identical math to v5 (kl = 4W/(T*Zu) + log Zv - log Zu;
    ce from Zce with host s[label] gather), now with CE_SCALE=1.

Semaphore ledger (g = 4q+k, aE: +1/ACT op, vD: +1/DVE op, dTS: +16/load):
  DMA(q):  vD >= 8q-2 (q>=1: W(4q-2), frees et/db scratch for A1(4q) via
           transitivity), aE >= 12q-12 (q>=2: A3(4q-5), tile q-2 ACT done)
  A1(g):   k==0: dTS >= 16(q+1)  (tile loaded; implies vD>=8q-2 via DMA)
           k>0:  vD >= 2g-2      (W(g-2) freed et[g%2]; tile load implied
                                  by A1(4q) program order)
  A2,A3:   no waits (program order after A1)
  D(g):    k==0: dTS >= 16(q+1); else program order
  W(g):    aE >= 3g+1            (A1(g) wrote et[g%2])
"""

import math
from contextlib import ExitStack

import numpy as np

import concourse.bass as bass
import concourse.mybir as mybir
from concourse.bass_utils import run_bass_kernel_spmd

ALPHA = 0.7
TEMP = 5.0
PAD_ID = 0
NUM_CHUNKS = 4

N_CORES = 8
B, S, V = 2, 2048, 32000
TOK = B * S                      # 4096 tokens total
TPC = TOK // N_CORES             # 512 tokens per core
P = 128                          # SBUF partitions
Q = TPC // P                     # 4 tiles per core (128 tokens each)
K = NUM_CHUNKS                   # 4 segments per tile
G = Q * K                        # 16 segments per core
CHW = V // NUM_CHUNKS            # 8000
PRESCALE = 0.25                  # host multiplies logits by this before fp8

F8 = mybir.dt.float8e3           # e3m4: max 15.5
BF16 = mybir.dt.bfloat16
F32 = mybir.dt.float32
EXP = mybir.ActivationFunctionType.Exp
MULT = mybir.AluOpType.mult
SUB = mybir.AluOpType.subtract
BYPASS = mybir.AluOpType.bypass

NSTAT = 4 * G                    # [W:0..15 | Zu:16..31 | Zv:32..47 | Zce:48..63]


def _build_nc(repeat=1):
    """Per-core program over ts=[2, TPC, V] fp8 (pre-scaled by 1/4)."""
    nc = bass.Bass()
    ts = nc.dram_tensor("ts", [2, TPC, V], F8, kind="ExternalInput")
    st = nc.dram_tensor("stats", [P, NSTAT], F32, kind="ExternalOutput")

    with ExitStack() as ctx:
        tiles = [
            ctx.enter_context(nc.sbuf_tensor(f"tile{i}", [P, 2 * V], F8))
            for i in range(2)
        ]
        et = [
            ctx.enter_context(nc.sbuf_tensor(f"et{i}", [P, CHW], F8))
            for i in range(2)
        ]
        db = [
            ctx.enter_context(nc.sbuf_tensor(f"db{i}", [P, CHW], F8))
            for i in range(2)
        ]
        junk = ctx.enter_context(nc.sbuf_tensor("junk", [P, CHW], BF16))
        junkv = ctx.enter_context(nc.sbuf_tensor("junkv", [P, CHW], F8))
        acc = ctx.enter_context(nc.sbuf_tensor("acc", [P, NSTAT], F32))
        dTS = ctx.enter_context(nc.semaphore("dTS"))
        aE = ctx.enter_context(nc.semaphore("aE"))
        vD = ctx.enter_context(nc.semaphore("vD"))
        out_sem = ctx.enter_context(nc.semaphore("out_sem"))
        block = ctx.enter_context(nc.Block())

        nq = Q * repeat
        ng = G * repeat

        def segs(g):
            q, k = (g % G) // K, g % K
            buf = tiles[q % 2]
            t_seg = buf[:, k * CHW:(k + 1) * CHW]
            s_seg = buf[:, V + k * CHW:V + (k + 1) * CHW]
            return t_seg, s_seg

        @block.sync
        def _(sync):
            for q in range(nq):
                if q >= 1:
                    sync.wait_ge(vD, 8 * q - 2)
                if q >= 2:
                    sync.wait_ge(aE, 12 * q - 12)
                r0 = (q % Q) * P
                src = ts[:, r0:r0 + P, :].rearrange("a p v -> p a v")
                dst = tiles[q % 2][:].rearrange("p (a v) -> p a v", a=2)
                sync.dma_start(out=dst, in_=src).then_inc(dTS, 16)
            sync.wait_ge(aE, 3 * ng)
            sync.wait_ge(vD, 2 * ng)
            sync.dma_start(out=st[:, :], in_=acc[:]).then_inc(out_sem, 16)
            sync.wait_ge(out_sem, 16)

        @block.scalar
        def _(scalar):
            for g in range(ng):
                q, k = g // K, g % K
                gg = g % G
                t_seg, s_seg = segs(g)
                if k == 0:
                    scalar.wait_ge(dTS, 16 * (q + 1))
                elif g >= 2:
                    scalar.wait_ge(vD, 2 * g - 2)
                nc.scalar.activation(
                    et[g % 2][:, :], t_seg, EXP, bias=0.0, scale=0.8,
                    accum_out=acc[:, G + gg:G + gg + 1],
                ).then_inc(aE, 1)
                nc.scalar.activation(
                    junk[:, :], s_seg, EXP, bias=0.0, scale=0.8,
                    accum_out=acc[:, 2 * G + gg:2 * G + gg + 1],
                ).then_inc(aE, 1)
                nc.scalar.activation(
                    junk[:, :], s_seg, EXP, bias=0.0, scale=4.0,
                    accum_out=acc[:, 3 * G + gg:3 * G + gg + 1],
                ).then_inc(aE, 1)

        @block.vector
        def _(vector):
            for g in range(ng):
                q, k = g // K, g % K
                gg = g % G
                t_seg, s_seg = segs(g)
                if k == 0:
                    vector.wait_ge(dTS, 16 * (q + 1))
                nc.vector.tensor_tensor(
                    out=db[g % 2][:, :], in0=t_seg, in1=s_seg, op=SUB,
                ).then_inc(vD, 1)
                vector.wait_ge(aE, 3 * g + 1)
                nc.vector.scalar_tensor_tensor(
                    out=junkv[:, :], in0=db[g % 2][:, :], scalar=1.0,
                    in1=et[g % 2][:, :], op0=BYPASS, op1=MULT,
                    accum_out=acc[:, gg:gg + 1],
                ).then_inc(vD, 1)

    return nc


_NC_CACHE = {}
last_results = None


def _get_nc(repeat=1):
    if repeat not in _NC_CACHE:
        _NC_CACHE[repeat] = _build_nc(repeat)
    return _NC_CACHE[repeat]


def _combine(results, s_full, lab):
    """Host-side float64 reduction of per-core [128, 64] stats -> loss."""
    # token = c*TPC + q*P + p ; segment gg = 4q + chunk j
    w = np.empty((TOK, NUM_CHUNKS))
    zu = np.empty((TOK, NUM_CHUNKS))
    zv = np.empty((TOK, NUM_CHUNKS))
    zce = np.empty(TOK)

    def tokmajor(block):  # [P, G] -> [TPC, NUM_CHUNKS] in token order
        return block.reshape(P, Q, K).transpose(1, 0, 2).reshape(TPC, K)

    for c, r in enumerate(results):
        a = r["stats"].astype(np.float64)          # [128, 64]
        sl = slice(c * TPC, (c + 1) * TPC)
        w[sl] = tokmajor(a[:, 0:G])
        zu[sl] = tokmajor(a[:, G:2 * G])
        zv[sl] = tokmajor(a[:, 2 * G:3 * G])
        zce[sl] = tokmajor(a[:, 3 * G:4 * G]).sum(axis=1)

    # W stored = sum e_t*(t-s)/4 -> true sum e_t*(t-s) = 4*W
    kl = (4.0 * w) / (TEMP * zu) + np.log(zv) - np.log(zu)
    total_kl = kl.sum() * (TEMP * TEMP) * (CHW / V) / B

    s_label = s_full[np.arange(TOK), lab].astype(np.float64)
    nll = np.log(zce) - s_label
    valid = lab != PAD_ID
    n_valid = max(int(valid.sum()), 1)
    ce = float(nll[valid].sum()) / n_valid

    return ALPHA * total_kl + (1.0 - ALPHA) * ce


def kernel(student_logits, teacher_logits, labels):
    global last_results
    np_f8 = mybir.dt.np(F8)
    s_full = np.asarray(student_logits, dtype=np.float32).reshape(TOK, V)
    t_full = np.asarray(teacher_logits, dtype=np.float32).reshape(TOK, V)
    lab = np.asarray(labels).reshape(TOK).astype(np.int64)
    s_f8 = (s_full * PRESCALE).astype(np_f8)
    t_f8 = (t_full * PRESCALE).astype(np_f8)

    nc = _get_nc()
    in_maps = []
    for c in range(N_CORES):
        ts = np.ascontiguousarray(np.stack(
            [t_f8[c * TPC:(c + 1) * TPC], s_f8[c * TPC:(c + 1) * TPC]], axis=0))
        in_maps.append({"ts": ts})
    last_results = run_bass_kernel_spmd(nc, in_maps, core_ids=list(range(N_CORES)))
    loss = _combine(last_results.results, s_full, lab)
    return np.array(loss, dtype=np.float32)


# revision 6
# speedup vs baseline: 4.1879x; 1.0488x over previous
"""Distillation loss (chunked KL + CE) on 8 Trainium2 NeuronCores — v6.

v5 (1.46ms cost-model timeline) was fully serialized: every wide op
depended on the previous one through a shared 3-buffer arena, so the
per-core time was the SUM of all engine busy times (ACT 320us + Pool
508us + DVE 534us + DMA 91us).

v6 restructures the math so the per-(token,chunk) sums come for free
from instruction-level accumulators, drops GPSIMD entirely, and
pipelines quarter-shard tiles so the Activation engine (the irreducible
3x exp streams) is the only wall:

  * per core: ts=[2, 512, 32000] fp8 (host pre-scales logits by 1/4);
    4 tiles of 128 tokens, one token per partition (full 32000-vocab row
    contiguous per partition), double-buffered DMA (2x32000B descriptors
    per partition per tile).
  * per segment g = (tile q, chunk k) = [128, 8000]:
      ACT A1: et = exp(0.8*t')        accum -> Zu(g)   (e_t = exp(t/5))
      ACT A2: junk = exp(0.8*s')      accum -> Zv(g)
      ACT A3: junk = exp(4*s') bf16   accum -> Zce(g)  (= exp(s), no
              bias needed: bf16 out cannot overflow like fp8)
      DVE D : db = t' - s'            (= (t-s)/4, fp8)
      DVE W : stt (db bypass 1) mult et, accum -> W'(g) (= sum e_t(t-s)/4)
  * accumulators are f32 sums of the pre-downcast activation outputs --
    strictly more precise than v5's fp8 tensor_reduce.
  * engine busy/core: ACT 336us (wall), DVE 271us, DMA 91us, Pool/PE 0.
  * host combine identical math to v5 (kl = 4W/(T*Zu) + log Zv - log Zu;
    ce from Zce with host s[label] gather), now with CE_SCALE=1.

Semaphore ledger (g = 4q+k, aE: +1/ACT op, vD: +1/DVE op, dTS: +16/load):
  DMA(q):  vD >= 8q-2 (q>=1: W(4q-2), frees et/db scratch for A1(4q) via
           transitivity), aE >= 12q-12 (q>=2: A3(4q-5), tile q-2 ACT done)
  A1(g):   k==0: dTS >= 16(q+1)  (tile loaded; implies vD>=8q-2 via DMA)
           k>0:  vD >= 2g-2      (W(g-2) freed et[g%2]; tile load implied
                                  by A1(4q) program order)
  A2,A3:   no waits (program order after A1)
  D(g):    k==0: dTS >= 16(q+1); else program order
  W(g):    aE >= 3g+1            (A1(g) wrote et[g%2])
"""

import math
from contextlib import ExitStack

import numpy as np

import concourse.bass as bass
import concourse.mybir as mybir
from concourse.bass_utils import run_bass_kernel_spmd

ALPHA = 0.7
TEMP = 5.0
PAD_ID = 0
NUM_CHUNKS = 4

N_CORES = 8
B, S, V = 2, 2048, 32000
TOK = B * S                      # 4096 tokens total
TPC = TOK // N_CORES             # 512 tokens per core
P = 128                          # SBUF partitions
Q = TPC // P                     # 4 tiles per core (128 tokens each)
K = NUM_CHUNKS                   # 4 segments per tile
G = Q * K                        # 16 segments per core
CHW = V // NUM_CHUNKS            # 8000
PRESCALE = 0.25                  # host multiplies logits by this before fp8

F8 = mybir.dt.float8e3           # e3m4: max 15.5
BF16 = mybir.dt.bfloat16
F32 = mybir.dt.float32
EXP = mybir.ActivationFunctionType.Exp
MULT = mybir.AluOpType.mult
SUB = mybir.AluOpType.subtract
BYPASS = mybir.AluOpType.bypass

NSTAT = 4 * G                    # [W:0..15 | Zu:16..31 | Zv:32..47 | Zce:48..63]


def _build_nc(repeat=1):
    """Per-core program over ts=[2, TPC, V] fp8 (pre-scaled by 1/4)."""
    nc = bass.Bass()
    ts = nc.dram_tensor("ts", [2, TPC, V], F8, kind="ExternalInput")
    st = nc.dram_tensor("stats", [P, NSTAT], F32, kind="ExternalOutput")

    with ExitStack() as ctx:
        tiles = [
            ctx.enter_context(nc.sbuf_tensor(f"tile{i}", [P, 2 * V], F8))
            for i in range(2)
        ]
        et = [
            ctx.enter_context(nc.sbuf_tensor(f"et{i}", [P, CHW], F8))
            for i in range(2)
        ]
        db = [
            ctx.enter_context(nc.sbuf_tensor(f"db{i}", [P, CHW], F8))
            for i in range(2)
        ]
        junk = ctx.enter_context(nc.sbuf_tensor("junk", [P, CHW], BF16))
        junkv = ctx.enter_context(nc.sbuf_tensor("junkv", [P, CHW], F8))
        acc = ctx.enter_context(nc.sbuf_tensor("acc", [P, NSTAT], F32))
        dTS = ctx.enter_context(nc.semaphore("dTS"))
        aE = ctx.enter_context(nc.semaphore("aE"))
        vD = ctx.enter_context(nc.semaphore("vD"))
        out_sem = ctx.enter_context(nc.semaphore("out_sem"))
        block = ctx.enter_context(nc.Block())

        nq = Q * repeat
        ng = G * repeat

        def segs(g):
            q, k = (g % G) // K, g % K
            buf = tiles[q % 2]
            t_seg = buf[:, k * CHW:(k + 1) * CHW]
            s_seg = buf[:, V + k * CHW:V + (k + 1) * CHW]
            return t_seg, s_seg

        @block.sync
        def _(sync):
            # Segment-granular loads: one dma_start per (tile, chunk) so the
            # first A1 can start after ~6us instead of a full-tile 23us.
            for g in range(ng):
                q, k = g // K, g % K
                if k == 0:
                    if q >= 1:
                        sync.wait_ge(vD, 8 * q - 2)
                    if q >= 2:
                        sync.wait_ge(aE, 12 * q - 12)
                r0 = (q % Q) * P
                src = ts[:, r0:r0 + P, k * CHW:(k + 1) * CHW].rearrange(
                    "a p v -> p a v")
                dst = tiles[q % 2][:].rearrange(
                    "p (a v) -> p a v", a=2)[:, :, k * CHW:(k + 1) * CHW]
                sync.dma_start(out=dst, in_=src).then_inc(dTS, 16)
            sync.wait_ge(aE, 3 * ng)
            sync.wait_ge(vD, 2 * ng)
            sync.dma_start(out=st[:, :], in_=acc[:]).then_inc(out_sem, 16)
            sync.wait_ge(out_sem, 16)

        @block.scalar
        def _(scalar):
            for g in range(ng):
                q, k = g // K, g % K
                gg = g % G
                t_seg, s_seg = segs(g)
                scalar.wait_ge(dTS, 16 * (g + 1))
                if k != 0 and g >= 2:
                    scalar.wait_ge(vD, 2 * g - 2)
                nc.scalar.activation(
                    et[g % 2][:, :], t_seg, EXP, bias=0.0, scale=0.8,
                    accum_out=acc[:, G + gg:G + gg + 1],
                ).then_inc(aE, 1)
                nc.scalar.activation(
                    junk[:, :], s_seg, EXP, bias=0.0, scale=0.8,
                    accum_out=acc[:, 2 * G + gg:2 * G + gg + 1],
                ).then_inc(aE, 1)
                nc.scalar.activation(
                    junk[:, :], s_seg, EXP, bias=0.0, scale=4.0,
                    accum_out=acc[:, 3 * G + gg:3 * G + gg + 1],
                ).then_inc(aE, 1)

        @block.vector
        def _(vector):
            for g in range(ng):
                q, k = g // K, g % K
                gg = g % G
                t_seg, s_seg = segs(g)
                vector.wait_ge(dTS, 16 * (g + 1))
                nc.vector.tensor_tensor(
                    out=db[g % 2][:, :], in0=t_seg, in1=s_seg, op=SUB,
                ).then_inc(vD, 1)
                vector.wait_ge(aE, 3 * g + 1)
                nc.vector.scalar_tensor_tensor(
                    out=junkv[:, :], in0=db[g % 2][:, :], scalar=1.0,
                    in1=et[g % 2][:, :], op0=BYPASS, op1=MULT,
                    accum_out=acc[:, gg:gg + 1],
                ).then_inc(vD, 1)

    return nc


_NC_CACHE = {}
last_results = None


def _get_nc(repeat=1):
    if repeat not in _NC_CACHE:
        _NC_CACHE[repeat] = _build_nc(repeat)
    return _NC_CACHE[repeat]


def _combine(results, s_full, lab):
    """Host-side float64 reduction of per-core [128, 64] stats -> loss."""
    # token = c*TPC + q*P + p ; segment gg = 4q + chunk j
    w = np.empty((TOK, NUM_CHUNKS))
    zu = np.empty((TOK, NUM_CHUNKS))
    zv = np.empty((TOK, NUM_CHUNKS))
    zce = np.empty(TOK)

    def tokmajor(block):  # [P, G] -> [TPC, NUM_CHUNKS] in token order
        return block.reshape(P, Q, K).transpose(1, 0, 2).reshape(TPC, K)

    for c, r in enumerate(results):
        a = r["stats"].astype(np.float64)          # [128, 64]
        sl = slice(c * TPC, (c + 1) * TPC)
        w[sl] = tokmajor(a[:, 0:G])
        zu[sl] = tokmajor(a[:, G:2 * G])
        zv[sl] = tokmajor(a[:, 2 * G:3 * G])
        zce[sl] = tokmajor(a[:, 3 * G:4 * G]).sum(axis=1)

    # W stored = sum e_t*(t-s)/4 -> true sum e_t*(t-s) = 4*W
    kl = (4.0 * w) / (TEMP * zu) + np.log(zv) - np.log(zu)
    total_kl = kl.sum() * (TEMP * TEMP) * (CHW / V) / B

    s_label = s_full[np.arange(TOK), lab].astype(np.float64)
    nll = np.log(zce) - s_label
    valid = lab != PAD_ID
    n_valid = max(int(valid.sum()), 1)
    ce = float(nll[valid].sum()) / n_valid

    return ALPHA * total_kl + (1.0 - ALPHA) * ce


def kernel(student_logits, teacher_logits, labels):
    global last_results
    np_f8 = mybir.dt.np(F8)
    s_full = np.asarray(student_logits, dtype=np.float32).reshape(TOK, V)
    t_full = np.asarray(teacher_logits, dtype=np.float32).reshape(TOK, V)
    lab = np.asarray(labels).reshape(TOK).astype(np.int64)
    s_f8 = (s_full * PRESCALE).astype(np_f8)
    t_f8 = (t_full * PRESCALE).astype(np_f8)

    nc = _get_nc()
    in_maps = []
    for c in range(N_CORES):
        ts = np.ascontiguousarray(np.stack(
            [t_f8[c * TPC:(c + 1) * TPC], s_f8[c * TPC:(c + 1) * TPC]], axis=0))
        in_maps.append({"ts": ts})
    last_results = run_bass_kernel_spmd(nc, in_maps, core_ids=list(range(N_CORES)))
    loss = _combine(last_results.results, s_full, lab)
    return np.array(loss, dtype=np.float32)


# revision 16
# speedup vs baseline: 4.7415x; 1.1322x over previous
"""Distillation loss (chunked KL + CE) on 8 Trainium2 NeuronCores — v7.

v6 (367us) made the Activation engine the only wall: 3 exp passes per
segment (e_t, e_s, e_ce) at 7.04us each, 48 instrs = 338us busy, with
DVE at 271us and Pool/PE idle.

v7 load-balances ALL THREE wide engines by giving each of the 16
segments (tile q of 128 tokens x chunk k of 8000 vocab) one of three
flavors:

  A  (5 segs): ACT A1,A2,A3 (Zu, Zv, Zce accums);  Pool: W1,W2 stt
  B1 (2 segs): ACT A1,A2; DVE: m2,m4,Zce-chain;    Pool: W1,W2 stt
  B2 (9 segs): ACT A1,A2; DVE: W1,W2,m2,m4;        Pool: Zce stt

where per segment (t', s' = logits/4 in fp8):
  A1: et  = exp(0.8 t') f8    accum -> Zu        (e_t = exp(t/5))
  A2: es  = exp(0.8 s') bf16  accum -> Zv
  A3: junk= exp(4 s')   bf16  accum -> Zce       (= sum exp(s), A only)
  W1: stt (et byp) mult t'    accum -> W1   [W = 4*(W1 - W2)]
  W2: stt (et byp) mult s'    accum -> W2   (no fp8-rounded t-s diff)
  m2 = es*es (bf16 2x tt), m4 = m2*m2, Zce = stt (m4 byp) mult es accum
       (= sum es^5 = sum exp(s) via 3 bf16 roundings, B only)

Engine busy/core: ACT 5*21.1+11*14.1 = 261us, DVE 2*17+9*25.4 = 263us,
Pool 5*22.2+2*22.2+9*11.1 = 255us -> balanced ~263us wall vs v6's 338.

Pipelining: segment-granular loads into a 3-deep ring of [128, 16000]
f8 seg buffers; et/es scratch 3-deep, m2/m4 2-deep (indexed by B-seg
ordinal). Semaphores: dTS +16/load, aE +1/ACT op, vD +1/DVE op,
pP +1/Pool op; prefix-sum arrays give exact wait values per segment.
Key transitive edges: A1(g) waits {dTS(g), vD/pP after seg g-3} which
frees et[g%3]/es[g%3]; DVE/Pool ops wait on aE for A1/A2 of their own
segment (implying the load); loads wait all three engines past seg g-3.
"""

import math
from contextlib import ExitStack

import numpy as np

import concourse.bass as bass
import concourse.mybir as mybir
from concourse.bass_utils import run_bass_kernel_spmd

ALPHA = 0.7
TEMP = 5.0
PAD_ID = 0
NUM_CHUNKS = 4

N_CORES = 8
B, S, V = 2, 2048, 32000
TOK = B * S                      # 4096 tokens total
TPC = TOK // N_CORES             # 512 tokens per core
P = 128                          # SBUF partitions
Q = TPC // P                     # 4 token tiles per core (128 tokens each)
K = NUM_CHUNKS                   # 4 segments per tile
G = Q * K                        # 16 segments per core
CHW = V // NUM_CHUNKS            # 8000
PRESCALE = 0.25                  # host multiplies logits by this before fp8

F8 = mybir.dt.float8e3
BF16 = mybir.dt.bfloat16
F32 = mybir.dt.float32
EXP = mybir.ActivationFunctionType.Exp
MULT = mybir.AluOpType.mult
SUB = mybir.AluOpType.subtract
BYPASS = mybir.AluOpType.bypass

# Flavor schedule (all ops verified compilable: Pool does only plain
# tensor_tensor; stt+accum lives on DVE; exp+accum on ACT):
#   A : ACT A1,A2,A3; Pool D=t-s;       DVE W-stt
#   AD: ACT A1,A2,A3;                   DVE W1,W2 stt (no D)
#   B : ACT A1,A2;    Pool D;           DVE m2,m4,Zstt,W
#   C : ACT A1,A2;    Pool D, m2;       DVE m4,Zstt,W
FLAV = ['B', 'C', 'A', 'B', 'A', 'B', 'B', 'A', 'A', 'B',
        'A', 'B', 'B', 'A', 'A', 'A']
assert len(FLAV) == G

A_FLAVS = ('A', 'AD')

# acc column layout: [Zu 0:16 | Zv 16:32 | W 32:48 | W2(AD) 48:64 | Zce 64:80]
NSTAT = 80


def _build_nc(repeat=1, flav_order=None):
    nc = bass.Bass()
    ts = nc.dram_tensor("ts", [2, TPC, V], F8, kind="ExternalInput")
    st = nc.dram_tensor("stats", [P, NSTAT], F32, kind="ExternalOutput")

    ng = G * repeat
    base_flav = flav_order if flav_order is not None else FLAV
    flav = [base_flav[g % G] for g in range(ng)]
    # B-seg ordinal (segments with a Zce chain) for es/m24 rings
    bidx = []
    b = 0
    for f in flav:
        bidx.append(b)
        if f not in A_FLAVS:
            b += 1
    bseg = [g for g in range(ng) if flav[g] not in A_FLAVS]

    # --- engine op streams (program order) + 1-based position maps -----
    # ACT: A1 runs one segment ahead of A2/A3.
    act_stream = [('A1', 0)]
    for g in range(ng):
        if g + 1 < ng:
            act_stream.append(('A1', g + 1))
        act_stream.append(('A2', g))
        if flav[g] in A_FLAVS:
            act_stream.append(('A3', g))

    # Pool: D(g) asap; for C segs also m2(g) (after A2(g)).
    pool_stream = []
    for g in range(ng):
        if flav[g] != 'AD':
            pool_stream.append(('D', g))
        if flav[g] == 'C':
            pool_stream.append(('m2', g))

    # DVE: A: [W]; AD: [W1, W2]; B: [m2, m4, Zstt, W]; C: [W, m4, Zstt]
    dve_stream = []
    for g in range(ng):
        f = flav[g]
        if f == 'A':
            dve_stream.append(('W', g))
        elif f == 'AD':
            dve_stream.append(('W1', g))
            dve_stream.append(('W2', g))
        elif f == 'B':
            dve_stream.extend([('m2', g), ('m4', g), ('Zstt', g), ('W', g)])
        else:  # C
            dve_stream.extend([('W', g), ('m4', g), ('Zstt', g)])

    def posmap(stream):
        m = {}
        for i, op in enumerate(stream):
            m[op] = i + 1
        return m

    aP = posmap(act_stream)
    pPos = posmap(pool_stream)
    vPos = posmap(dve_stream)
    aE_total, pP_total, vD_total = len(act_stream), len(pool_stream), len(dve_stream)

    def after_last_act(g):      # last ACT op of seg g
        return aP[('A3', g)] if flav[g] in A_FLAVS else aP[('A2', g)]

    def after_last_dve_w(g):    # the W product(s) of seg g on DVE
        return vPos[('W2', g)] if flav[g] == 'AD' else vPos[('W', g)]

    def waits_readers_of_bufs(g):
        """t/s buffer of seg g is free when ACT (A2/A3), Pool (D) and
        DVE (W, in-place over the t-region) are all past seg g."""
        aw = after_last_act(g)
        vw = after_last_dve_w(g)
        pw = pPos[('D', g)] if flav[g] != 'AD' else 0
        return aw, vw, pw

    def waits_readers_of_et(g):
        return after_last_dve_w(g), 0

    def waits_readers_of_es(g):
        """Readers of the es slot written at seg g (B/C chains only)."""
        vw, pw = 0, 0
        if flav[g] == 'B':
            vw = vPos[('Zstt', g)]
        elif flav[g] == 'C':
            vw = vPos[('Zstt', g)]
            pw = pPos[('m2', g)]
        return vw, pw

    with ExitStack() as ctx:
        bufs = [
            ctx.enter_context(nc.sbuf_tensor(f"buf{i}", [P, 2 * CHW], F8))
            for i in range(4)
        ]
        et = [
            ctx.enter_context(nc.sbuf_tensor(f"et{i}", [P, CHW], F8))
            for i in range(4)
        ]
        es = [
            ctx.enter_context(nc.sbuf_tensor(f"es{i}", [P, CHW], BF16))
            for i in range(3)
        ]
        m24 = [
            ctx.enter_context(nc.sbuf_tensor(f"m24_{i}", [P, CHW], BF16))
            for i in range(3)
        ]
        acc = ctx.enter_context(nc.sbuf_tensor("acc", [P, NSTAT], F32))
        dTS = ctx.enter_context(nc.semaphore("dTS"))
        aE = ctx.enter_context(nc.semaphore("aE"))
        vD = ctx.enter_context(nc.semaphore("vD"))
        pP = ctx.enter_context(nc.semaphore("pP"))
        out_sem = ctx.enter_context(nc.semaphore("out_sem"))
        block = ctx.enter_context(nc.Block())

        def tseg(g):
            return bufs[g % 4][:, 0:CHW]

        def sseg(g):
            return bufs[g % 4][:, CHW:2 * CHW]

        def etb(g):
            return et[g % 4][:, :]

        def esb(g):
            return es[bidx[g] % 3][:, :]

        def m24b(g):
            return m24[bidx[g] % 3][:, :]

        def col(base, g):
            c = base * G + (g % G)
            return acc[:, c:c + 1]

        @block.sync
        def _(sync):
            for g in range(ng):
                q, k = (g % G) // K, g % K
                if g >= 4:
                    aw, vw, pw = waits_readers_of_bufs(g - 4)
                    sync.wait_ge(aE, aw)
                    sync.wait_ge(vD, vw)
                    if pw > 0:
                        sync.wait_ge(pP, pw)
                r0 = q * P
                src = ts[:, r0:r0 + P, k * CHW:(k + 1) * CHW].rearrange(
                    "a p v -> p a v")
                dst = bufs[g % 4][:].rearrange("p (a v) -> p a v", a=2)
                sync.dma_start(out=dst, in_=src).then_inc(dTS, 16)
            sync.wait_ge(aE, aE_total)
            sync.wait_ge(vD, vD_total)
            sync.wait_ge(pP, pP_total)
            sync.dma_start(out=st[:, :], in_=acc[:]).then_inc(out_sem, 16)
            sync.wait_ge(out_sem, 16)

        @block.scalar
        def _(scalar):
            for kind, g in act_stream:
                if kind == 'A1':
                    scalar.wait_ge(dTS, 16 * (g + 1))
                    if g >= 4:
                        vw, pw = waits_readers_of_et(g - 4)
                        scalar.wait_ge(vD, vw)
                    nc.scalar.activation(
                        etb(g), tseg(g), EXP, bias=0.0, scale=0.8,
                        accum_out=col(0, g),
                    ).then_inc(aE, 1)
                elif kind == 'A2':
                    bprev = bidx[g] - 3   # es ring depth 3
                    if bprev >= 0:
                        vw, pw = waits_readers_of_es(bseg[bprev])
                        if vw > 0:
                            scalar.wait_ge(vD, vw)
                        if pw > 0:
                            scalar.wait_ge(pP, pw)
                    nc.scalar.activation(
                        esb(g), sseg(g), EXP, bias=0.0, scale=0.8,
                        accum_out=col(1, g),
                    ).then_inc(aE, 1)
                else:  # A3
                    nc.scalar.activation(
                        esb(g), sseg(g), EXP, bias=0.0, scale=4.0,
                        accum_out=col(4, g),
                    ).then_inc(aE, 1)

        @block.vector
        def _(vector):
            for kind, g in dve_stream:
                if kind == 'W':
                    # (D byp 1) mult e_t, in place over the t-region.
                    # D(g) on Pool implies A1(g) (its own wait) and load.
                    vector.wait_ge(pP, pPos[('D', g)])
                    nc.vector.scalar_tensor_tensor(
                        out=tseg(g), in0=tseg(g), scalar=1.0,
                        in1=etb(g), op0=BYPASS, op1=MULT,
                        accum_out=col(2, g),
                    ).then_inc(vD, 1)
                elif kind == 'W1':
                    vector.wait_ge(aE, aP[('A1', g)])
                    nc.vector.scalar_tensor_tensor(
                        out=tseg(g), in0=etb(g), scalar=1.0,
                        in1=tseg(g), op0=BYPASS, op1=MULT,
                        accum_out=col(2, g),
                    ).then_inc(vD, 1)
                elif kind == 'W2':
                    nc.vector.scalar_tensor_tensor(
                        out=tseg(g), in0=etb(g), scalar=1.0,
                        in1=sseg(g), op0=BYPASS, op1=MULT,
                        accum_out=col(3, g),
                    ).then_inc(vD, 1)
                elif kind == 'm2':   # B only (C's m2 is on Pool)
                    vector.wait_ge(aE, aP[('A2', g)])
                    nc.vector.tensor_tensor(
                        out=m24b(g), in0=esb(g), in1=esb(g), op=MULT,
                    ).then_inc(vD, 1)
                elif kind == 'm4':
                    if flav[g] == 'C':
                        vector.wait_ge(pP, pPos[('m2', g)])
                    nc.vector.tensor_tensor(
                        out=m24b(g), in0=m24b(g), in1=m24b(g), op=MULT,
                    ).then_inc(vD, 1)
                else:  # Zstt
                    nc.vector.scalar_tensor_tensor(
                        out=m24b(g), in0=m24b(g), scalar=1.0,
                        in1=esb(g), op0=BYPASS, op1=MULT,
                        accum_out=col(4, g),
                    ).then_inc(vD, 1)

        @block.gpsimd
        def _(gp):
            for kind, g in pool_stream:
                if kind == 'D':
                    # D = t - s in place over the t-region (A1(g) read t)
                    gp.wait_ge(aE, aP[('A1', g)])
                    nc.gpsimd.tensor_tensor(
                        out=tseg(g), in0=tseg(g), in1=sseg(g), op=SUB,
                    ).then_inc(pP, 1)
                else:  # m2 for C segs
                    gp.wait_ge(aE, aP[('A2', g)])
                    bprev = bidx[g] - 3
                    if bprev >= 0:
                        f2 = flav[bseg[bprev]]
                        if f2 in ('B', 'C'):
                            gp.wait_ge(vD, vPos[('Zstt', bseg[bprev])])
                    nc.gpsimd.tensor_tensor(
                        out=m24b(g), in0=esb(g), in1=esb(g), op=MULT,
                    ).then_inc(pP, 1)

    return nc


_NC_CACHE = {}
last_results = None


def _get_nc(repeat=1):
    if repeat not in _NC_CACHE:
        _NC_CACHE[repeat] = _build_nc(repeat)
    return _NC_CACHE[repeat]


def _combine(results, s_full, lab):
    """Host-side float64 reduction of per-core [128, 80] stats -> loss."""
    # token = c*TPC + q*P + p ; segment g = 4q + chunk j
    w = np.empty((TOK, NUM_CHUNKS))
    zu = np.empty((TOK, NUM_CHUNKS))
    zv = np.empty((TOK, NUM_CHUNKS))
    zce = np.empty(TOK)

    def tokmajor(block):  # [P, G] -> [TPC, NUM_CHUNKS] in token order
        return block.reshape(P, Q, K).transpose(1, 0, 2).reshape(TPC, K)

    for c, r in enumerate(results):
        a = r["stats"].astype(np.float64)          # [128, 80]
        sl = slice(c * TPC, (c + 1) * TPC)
        zu[sl] = tokmajor(a[:, 0:G])
        zv[sl] = tokmajor(a[:, G:2 * G])
        wc = a[:, 2 * G:3 * G].copy()
        ad = np.array([f == 'AD' for f in FLAV])[None, :]
        wc = np.where(ad, wc - a[:, 3 * G:4 * G], wc)  # AD segs: W1 - W2
        w[sl] = tokmajor(wc)
        zce[sl] = tokmajor(a[:, 4 * G:5 * G]).sum(axis=1)

    # W stored = sum e_t*(t-s)/4 -> true sum e_t*(t-s) = 4*W
    kl = (4.0 * w) / (TEMP * zu) + np.log(zv) - np.log(zu)
    total_kl = kl.sum() * (TEMP * TEMP) * (CHW / V) / B

    s_label = s_full[np.arange(TOK), lab].astype(np.float64)
    nll = np.log(zce) - s_label
    valid = lab != PAD_ID
    n_valid = max(int(valid.sum()), 1)
    ce = float(nll[valid].sum()) / n_valid

    return ALPHA * total_kl + (1.0 - ALPHA) * ce


def kernel(student_logits, teacher_logits, labels):
    global last_results
    np_f8 = mybir.dt.np(F8)
    s_full = np.asarray(student_logits, dtype=np.float32).reshape(TOK, V)
    t_full = np.asarray(teacher_logits, dtype=np.float32).reshape(TOK, V)
    lab = np.asarray(labels).reshape(TOK).astype(np.int64)
    s_f8 = (s_full * PRESCALE).astype(np_f8)
    t_f8 = (t_full * PRESCALE).astype(np_f8)

    nc = _get_nc()
    in_maps = []
    for c in range(N_CORES):
        ts = np.ascontiguousarray(np.stack(
            [t_f8[c * TPC:(c + 1) * TPC], s_f8[c * TPC:(c + 1) * TPC]], axis=0))
        in_maps.append({"ts": ts})
    last_results = run_bass_kernel_spmd(nc, in_maps, core_ids=list(range(N_CORES)))
    loss = _combine(last_results.results, s_full, lab)
    return np.array(loss, dtype=np.float32)


# revision 29
# speedup vs baseline: 4.9995x; 1.0544x over previous
"""Distillation loss (chunked KL + CE) on 8 Trainium2 NeuronCores — v7.

v6 (367us) made the Activation engine the only wall: 3 exp passes per
segment (e_t, e_s, e_ce) at 7.04us each, 48 instrs = 338us busy, with
DVE at 271us and Pool/PE idle.

v7 load-balances ALL THREE wide engines by giving each of the 16
segments (tile q of 128 tokens x chunk k of 8000 vocab) one of three
flavors:

  A  (5 segs): ACT A1,A2,A3 (Zu, Zv, Zce accums);  Pool: W1,W2 stt
  B1 (2 segs): ACT A1,A2; DVE: m2,m4,Zce-chain;    Pool: W1,W2 stt
  B2 (9 segs): ACT A1,A2; DVE: W1,W2,m2,m4;        Pool: Zce stt

where per segment (t', s' = logits/4 in fp8):
  A1: et  = exp(0.8 t') f8    accum -> Zu        (e_t = exp(t/5))
  A2: es  = exp(0.8 s') bf16  accum -> Zv
  A3: junk= exp(4 s')   bf16  accum -> Zce       (= sum exp(s), A only)
  W1: stt (et byp) mult t'    accum -> W1   [W = 4*(W1 - W2)]
  W2: stt (et byp) mult s'    accum -> W2   (no fp8-rounded t-s diff)
  m2 = es*es (bf16 2x tt), m4 = m2*m2, Zce = stt (m4 byp) mult es accum
       (= sum es^5 = sum exp(s) via 3 bf16 roundings, B only)

Engine busy/core: ACT 5*21.1+11*14.1 = 261us, DVE 2*17+9*25.4 = 263us,
Pool 5*22.2+2*22.2+9*11.1 = 255us -> balanced ~263us wall vs v6's 338.

Pipelining: segment-granular loads into a 3-deep ring of [128, 16000]
f8 seg buffers; et/es scratch 3-deep, m2/m4 2-deep (indexed by B-seg
ordinal). Semaphores: dTS +16/load, aE +1/ACT op, vD +1/DVE op,
pP +1/Pool op; prefix-sum arrays give exact wait values per segment.
Key transitive edges: A1(g) waits {dTS(g), vD/pP after seg g-3} which
frees et[g%3]/es[g%3]; DVE/Pool ops wait on aE for A1/A2 of their own
segment (implying the load); loads wait all three engines past seg g-3.
"""

from contextlib import ExitStack

import numpy as np

import concourse.bass as bass
import concourse.mybir as mybir
from concourse.bass_utils import run_bass_kernel_spmd

ALPHA = 0.7
TEMP = 5.0
PAD_ID = 0
NUM_CHUNKS = 4

N_CORES = 8
B, S, V = 2, 2048, 32000
TOK = B * S                      # 4096 tokens total
TPC = TOK // N_CORES             # 512 tokens per core
P = 128                          # SBUF partitions
Q = TPC // P                     # 4 token tiles per core (128 tokens each)
K = NUM_CHUNKS                   # 4 segments per tile
G = Q * K                        # 16 segments per core
CHW = V // NUM_CHUNKS            # 8000
PRESCALE = 0.25                  # host multiplies logits by this before fp8

F8 = mybir.dt.float8e3
BF16 = mybir.dt.bfloat16
F32 = mybir.dt.float32
EXP = mybir.ActivationFunctionType.Exp
MULT = mybir.AluOpType.mult
SUB = mybir.AluOpType.subtract
BYPASS = mybir.AluOpType.bypass

# Flavor schedule (all ops verified compilable: Pool does only plain
# tensor_tensor; stt+accum lives on DVE; exp+accum on ACT):
#   A : ACT A1,A2,A3; Pool D=t-s;       DVE W-stt
#   AD: ACT A1,A2,A3;                   DVE W1,W2 stt (no D)
#   B : ACT A1,A2;    Pool D;           DVE m2,m4,Zstt,W
#   C : ACT A1,A2;    Pool D, m2;       DVE m4,Zstt,W
FLAV = ['B', 'B', 'A', 'B', 'A', 'B', 'A', 'B', 'B', 'A',
        'B', 'A', 'B', 'A', 'A', 'A']
assert len(FLAV) == G

A_FLAVS = ('A', 'AD')

# acc column layout: [Zu 0:16 | Zv 16:32 | W 32:48 | W2(AD) 48:64 | Zce 64:80]
NSTAT = 80


def _build_nc(repeat=1, flav_order=None):
    nc = bass.Bass()
    ts = nc.dram_tensor("ts", [2, TPC, V], F8, kind="ExternalInput")
    st = nc.dram_tensor("stats", [P, NSTAT], F32, kind="ExternalOutput")

    ng = G * repeat
    base_flav = flav_order if flav_order is not None else FLAV
    flav = [base_flav[g % G] for g in range(ng)]
    # B-seg ordinal (segments with a Zce chain) for es/m24 rings
    bidx = []
    b = 0
    for f in flav:
        bidx.append(b)
        if f not in A_FLAVS:
            b += 1
    bseg = [g for g in range(ng) if flav[g] not in A_FLAVS]

    # --- engine op streams (program order) + 1-based position maps -----
    # ACT: A1 runs one segment ahead of A2/A3, but the cadence starts at
    # seg 1 so A2(0) lands second and the DVE/Pool chains ramp early.
    act_stream = [('A1', 0), ('A2', 0)]
    if flav[0] in A_FLAVS:
        act_stream.append(('A3', 0))
    if ng > 1:
        act_stream.append(('A1', 1))
    for g in range(1, ng):
        if g + 1 < ng:
            act_stream.append(('A1', g + 1))
        act_stream.append(('A2', g))
        if flav[g] in A_FLAVS:
            act_stream.append(('A3', g))

    # Pool: D(g) asap; for C segs also m2(g) (after A2(g)).
    pool_stream = []
    for g in range(ng):
        if flav[g] != 'AD':
            pool_stream.append(('D', g))
        if flav[g] == 'C':
            pool_stream.append(('m2', g))

    # DVE: A: [W]; AD: [W1, W2]; B: [m2, m4, Zstt, W]; C: [W, m4, Zstt]
    dve_stream = []
    for g in range(ng):
        f = flav[g]
        if f == 'A':
            dve_stream.append(('W', g))
        elif f == 'AD':
            dve_stream.append(('W1', g))
            dve_stream.append(('W2', g))
        elif f == 'B':
            dve_stream.extend([('m2', g), ('m4', g), ('Zstt', g), ('W', g)])
        else:  # C
            dve_stream.extend([('W', g), ('m4', g), ('Zstt', g)])

    def posmap(stream):
        m = {}
        for i, op in enumerate(stream):
            m[op] = i + 1
        return m

    aP = posmap(act_stream)
    pPos = posmap(pool_stream)
    vPos = posmap(dve_stream)
    aE_total, pP_total, vD_total = len(act_stream), len(pool_stream), len(dve_stream)

    def after_last_act(g):      # last ACT op of seg g
        return aP[('A3', g)] if flav[g] in A_FLAVS else aP[('A2', g)]

    def after_last_dve_w(g):    # the W product(s) of seg g on DVE
        return vPos[('W2', g)] if flav[g] == 'AD' else vPos[('W', g)]

    def waits_readers_of_bufs(g):
        """t/s buffer of seg g is free when ACT (A2/A3), Pool (D) and
        DVE (W, in-place over the t-region) are all past seg g."""
        aw = after_last_act(g)
        vw = after_last_dve_w(g)
        pw = pPos[('D', g)] if flav[g] != 'AD' else 0
        return aw, vw, pw

    def waits_readers_of_et(g):
        return after_last_dve_w(g), 0

    def waits_readers_of_es(g):
        """Readers of the es slot written at seg g (B/C chains only)."""
        vw, pw = 0, 0
        if flav[g] == 'B':
            vw = vPos[('Zstt', g)]
        elif flav[g] == 'C':
            vw = vPos[('Zstt', g)]
            pw = pPos[('m2', g)]
        return vw, pw

    with ExitStack() as ctx:
        bufs = [
            ctx.enter_context(nc.sbuf_tensor(f"buf{i}", [P, 2 * CHW], F8))
            for i in range(5)
        ]
        et = [
            ctx.enter_context(nc.sbuf_tensor(f"et{i}", [P, CHW], F8))
            for i in range(4)
        ]
        es = [
            ctx.enter_context(nc.sbuf_tensor(f"es{i}", [P, CHW], BF16))
            for i in range(3)
        ]
        m24 = [
            ctx.enter_context(nc.sbuf_tensor(f"m24_{i}", [P, CHW], BF16))
            for i in range(2)
        ]
        acc = ctx.enter_context(nc.sbuf_tensor("acc", [P, NSTAT], F32))
        dTS = ctx.enter_context(nc.semaphore("dTS"))
        aE = ctx.enter_context(nc.semaphore("aE"))
        vD = ctx.enter_context(nc.semaphore("vD"))
        pP = ctx.enter_context(nc.semaphore("pP"))
        out_sem = ctx.enter_context(nc.semaphore("out_sem"))
        block = ctx.enter_context(nc.Block())

        def tseg(g):
            return bufs[g % 5][:, 0:CHW]

        def sseg(g):
            return bufs[g % 5][:, CHW:2 * CHW]

        def etb(g):
            return et[g % 4][:, :]

        def esb(g):
            return es[bidx[g] % 3][:, :]

        def m24b(g):
            return m24[bidx[g] % 2][:, :]

        def col(base, g):
            c = base * G + (g % G)
            return acc[:, c:c + 1]

        @block.sync
        def _(sync):
            for g in range(ng):
                q, k = (g % G) // K, g % K
                if g >= 5:
                    aw, vw, pw = waits_readers_of_bufs(g - 5)
                    sync.wait_ge(aE, aw)
                    sync.wait_ge(vD, vw)
                    if pw > 0:
                        sync.wait_ge(pP, pw)
                r0 = q * P
                src = ts[:, r0:r0 + P, k * CHW:(k + 1) * CHW].rearrange(
                    "a p v -> p a v")
                dst = bufs[g % 5][:].rearrange("p (a v) -> p a v", a=2)
                if g == 0:
                    # split the very first load: t-half lands ~3us sooner
                    # so A1(0) starts earlier (halves inc +8 each)
                    sync.dma_start(out=dst[:, 0:1, :],
                                   in_=src[:, 0:1, :]).then_inc(dTS, 16)
                    sync.dma_start(out=dst[:, 1:2, :],
                                   in_=src[:, 1:2, :]).then_inc(dTS, 16)
                else:
                    sync.dma_start(out=dst, in_=src).then_inc(dTS, 16)
            sync.wait_ge(aE, aE_total)
            sync.wait_ge(vD, vD_total)
            sync.wait_ge(pP, pP_total)
            sync.dma_start(out=st[:, :], in_=acc[:]).then_inc(out_sem, 16)
            sync.wait_ge(out_sem, 16)

        @block.scalar
        def _(scalar):
            for kind, g in act_stream:
                if kind == 'A1':
                    scalar.wait_ge(dTS, 16 if g == 0 else 16 * (g + 2))
                    if g >= 4:
                        vw, pw = waits_readers_of_et(g - 4)
                        scalar.wait_ge(vD, vw)
                    nc.scalar.activation(
                        etb(g), tseg(g), EXP, bias=0.0, scale=0.8,
                        accum_out=col(0, g),
                    ).then_inc(aE, 1)
                elif kind == 'A2':
                    if g == 0:
                        scalar.wait_ge(dTS, 32)  # s-half of split load 0
                    bprev = bidx[g] - 3   # es ring depth 3
                    if bprev >= 0:
                        vw, pw = waits_readers_of_es(bseg[bprev])
                        if vw > 0:
                            scalar.wait_ge(vD, vw)
                        if pw > 0:
                            scalar.wait_ge(pP, pw)
                    nc.scalar.activation(
                        esb(g), sseg(g), EXP, bias=0.0, scale=0.8,
                        accum_out=col(1, g),
                    ).then_inc(aE, 1)
                else:  # A3
                    nc.scalar.activation(
                        esb(g), sseg(g), EXP, bias=0.0, scale=4.0,
                        accum_out=col(4, g),
                    ).then_inc(aE, 1)

        @block.vector
        def _(vector):
            for kind, g in dve_stream:
                if kind == 'W':
                    # (D byp 1) mult e_t, in place over the t-region.
                    # D(g) on Pool implies A1(g) (its own wait) and load.
                    vector.wait_ge(pP, pPos[('D', g)])
                    nc.vector.scalar_tensor_tensor(
                        out=tseg(g), in0=tseg(g), scalar=1.0,
                        in1=etb(g), op0=BYPASS, op1=MULT,
                        accum_out=col(2, g),
                    ).then_inc(vD, 1)
                elif kind == 'W1':
                    vector.wait_ge(aE, aP[('A1', g)])
                    nc.vector.scalar_tensor_tensor(
                        out=tseg(g), in0=etb(g), scalar=1.0,
                        in1=tseg(g), op0=BYPASS, op1=MULT,
                        accum_out=col(2, g),
                    ).then_inc(vD, 1)
                elif kind == 'W2':
                    nc.vector.scalar_tensor_tensor(
                        out=tseg(g), in0=etb(g), scalar=1.0,
                        in1=sseg(g), op0=BYPASS, op1=MULT,
                        accum_out=col(3, g),
                    ).then_inc(vD, 1)
                elif kind == 'm2':   # B only (C's m2 is on Pool)
                    vector.wait_ge(aE, aP[('A2', g)])
                    bp2 = bidx[g] - 2
                    if bp2 >= 0 and flav[bseg[bp2]] == 'C':
                        vector.wait_ge(pP, pPos[('m2', bseg[bp2])])
                    nc.vector.tensor_tensor(
                        out=m24b(g), in0=esb(g), in1=esb(g), op=MULT,
                    ).then_inc(vD, 1)
                elif kind == 'm4':
                    if flav[g] == 'C':
                        vector.wait_ge(pP, pPos[('m2', g)])
                    nc.vector.tensor_tensor(
                        out=m24b(g), in0=m24b(g), in1=m24b(g), op=MULT,
                    ).then_inc(vD, 1)
                else:  # Zstt
                    nc.vector.scalar_tensor_tensor(
                        out=m24b(g), in0=m24b(g), scalar=1.0,
                        in1=esb(g), op0=BYPASS, op1=MULT,
                        accum_out=col(4, g),
                    ).then_inc(vD, 1)

        @block.gpsimd
        def _(gp):
            for kind, g in pool_stream:
                if kind == 'D':
                    # D = t - s in place over the t-region (A1(g) read t)
                    if g == 0:
                        gp.wait_ge(dTS, 32)  # s-half of split load 0
                    gp.wait_ge(aE, aP[('A1', g)])
                    nc.gpsimd.tensor_tensor(
                        out=tseg(g), in0=tseg(g), in1=sseg(g), op=SUB,
                    ).then_inc(pP, 1)
                else:  # m2 for C segs
                    gp.wait_ge(aE, aP[('A2', g)])
                    bprev = bidx[g] - 2
                    if bprev >= 0:
                        f2 = flav[bseg[bprev]]
                        if f2 in ('B', 'C'):
                            gp.wait_ge(vD, vPos[('Zstt', bseg[bprev])])
                    nc.gpsimd.tensor_tensor(
                        out=m24b(g), in0=esb(g), in1=esb(g), op=MULT,
                    ).then_inc(pP, 1)

    return nc


_NC_CACHE = {}
last_results = None


def _get_nc(repeat=1):
    if repeat not in _NC_CACHE:
        _NC_CACHE[repeat] = _build_nc(repeat)
    return _NC_CACHE[repeat]


def _combine(results, s_full, lab):
    """Host-side float64 reduction of per-core [128, 80] stats -> loss."""
    # token = c*TPC + q*P + p ; segment g = 4q + chunk j
    w = np.empty((TOK, NUM_CHUNKS))
    zu = np.empty((TOK, NUM_CHUNKS))
    zv = np.empty((TOK, NUM_CHUNKS))
    zce = np.empty(TOK)

    def tokmajor(block):  # [P, G] -> [TPC, NUM_CHUNKS] in token order
        return block.reshape(P, Q, K).transpose(1, 0, 2).reshape(TPC, K)

    for c, r in enumerate(results):
        a = r["stats"].astype(np.float64)          # [128, 80]
        sl = slice(c * TPC, (c + 1) * TPC)
        zu[sl] = tokmajor(a[:, 0:G])
        zv[sl] = tokmajor(a[:, G:2 * G])
        wc = a[:, 2 * G:3 * G].copy()
        ad = np.array([f == 'AD' for f in FLAV])[None, :]
        wc = np.where(ad, wc - a[:, 3 * G:4 * G], wc)  # AD segs: W1 - W2
        w[sl] = tokmajor(wc)
        zce[sl] = tokmajor(a[:, 4 * G:5 * G]).sum(axis=1)

    # W stored = sum e_t*(t-s)/4 -> true sum e_t*(t-s) = 4*W
    kl = (4.0 * w) / (TEMP * zu) + np.log(zv) - np.log(zu)
    total_kl = kl.sum() * (TEMP * TEMP) * (CHW / V) / B

    s_label = s_full[np.arange(TOK), lab].astype(np.float64)
    nll = np.log(zce) - s_label
    valid = lab != PAD_ID
    n_valid = max(int(valid.sum()), 1)
    ce = float(nll[valid].sum()) / n_valid

    return ALPHA * total_kl + (1.0 - ALPHA) * ce


def kernel(student_logits, teacher_logits, labels):
    global last_results
    np_f8 = mybir.dt.np(F8)
    s_full = np.asarray(student_logits, dtype=np.float32).reshape(TOK, V)
    t_full = np.asarray(teacher_logits, dtype=np.float32).reshape(TOK, V)
    lab = np.asarray(labels).reshape(TOK).astype(np.int64)
    s_f8 = (s_full * PRESCALE).astype(np_f8)
    t_f8 = (t_full * PRESCALE).astype(np_f8)

    nc = _get_nc()
    in_maps = []
    for c in range(N_CORES):
        ts = np.ascontiguousarray(np.stack(
            [t_f8[c * TPC:(c + 1) * TPC], s_f8[c * TPC:(c + 1) * TPC]], axis=0))
        in_maps.append({"ts": ts})
    last_results = run_bass_kernel_spmd(nc, in_maps, core_ids=list(range(N_CORES)))
    loss = _combine(last_results.results, s_full, lab)
    return np.array(loss, dtype=np.float32)
